# revision 40
# baseline (speedup 1.0000x reference)
"""Trainium2 Bass kernel for nn_MPCActor: MLP (256->512->512->32, relu/relu/
sigmoid) + 100 SGD steps on u, solved in closed form (u <- a*u + b with
a = 1-2*lr*q_u has exact solution u_N = a^N u0 - 0.5*(p_u/q_u)*(1 - a^N)).

Data parallel over 8 NeuronCores: batch 32768 -> 4096 rows/core, weights
replicated. All matmul operands are fp8(e4m3): obs is pre-transposed and
pre-quantized on the host to [256, 4096] per core (feature-on-partition, so
no on-chip transposes), weights are scaled x64 on the host to dodge the fp8
subnormal cliff (undone in the drain scale) and packed in the DoubleRow
[K=128, 2, M] pair layout so each matmul contracts 256 rows per pass.

Layer 3 computes only the 8 useful W3 columns (q_u, p_u), zero-padded to 32
and col-tiled via tile_position so four batch tiles land concurrently in one
PSUM bank at partition offsets 0/32/64/96; one sigmoid drains all four, and
four [128,128] bf16 PE transposes flip a whole 4-tile round to batch-major.
The closed-form update then runs once per round on [128,4,4,4] APs.

PSUM drains alternate ACT/DVE; the closed-form's SBUF-only power chain runs
on the otherwise idle GpSimd. When the MLP biases are nonzero the kernel
falls back to per-chunk drains (ACT with exact bias; DVE/GpSimd chunks use
max(z,-64b)/64 whose constant deficit is folded into the next layer's bias
on the host).
"""

import numpy as np
import ml_dtypes

import concourse.bass as bass
import concourse.mybir as mybir
import concourse.tile as tile
from concourse import bacc, masks
from concourse.bass_utils import run_bass_kernel_spmd

NCORES = 8
BATCH = 32768
BPC = BATCH // NCORES  # 4096
OBS = 256
HID = 512
BT = 512               # batch tile (matmul moving free dim)
NT = BPC // BT         # 8 batch tiles per core
NR = NT // 4           # rounds of 4 tiles for layer 3
LR = 0.01
WS = 64.0              # weight pre-scale (host), undone in drain scale
F32 = mybir.dt.float32
BF16 = mybir.dt.bfloat16
FP8 = mybir.dt.float8e4
NPF8 = ml_dtypes.float8_e4m3

_CACHE = {}


def _build_nc(zero_bias: bool):
    nc = bacc.Bacc(
        trn_type="TRN2", target_bir_lowering=False, debug=False, num_devices=NCORES
    )
    obsd = nc.declare_dram_parameter("obsd", [OBS, BPC], FP8, isOutput=False).ap()
    w1d = nc.declare_dram_parameter("w1d", [128, 2, HID], FP8, isOutput=False).ap()
    w2d = nc.declare_dram_parameter("w2d", [128, 4, HID], FP8, isOutput=False).ap()
    w3d = nc.declare_dram_parameter("w3d", [128, 4, 32], FP8, isOutput=False).ap()
    bd = nc.declare_dram_parameter("bd", [128, 9], F32, isOutput=False).ap()
    u0d = nc.declare_dram_parameter("u0d", [NR, 128, 64], F32, isOutput=False).ap()
    uod = nc.declare_dram_parameter("uod", [NR, 128, 64], F32, isOutput=True).ap()

    AF = mybir.ActivationFunctionType
    ALU = mybir.AluOpType
    PM = mybir.MatmulPerfMode

    with tile.TileContext(nc) as tc:
        from contextlib import ExitStack

        with ExitStack() as ctx:
            singles = ctx.enter_context(tc.tile_pool(name="singles", bufs=1))
            p_obs = ctx.enter_context(tc.tile_pool(name="obs", bufs=4))
            p_y1 = ctx.enter_context(tc.tile_pool(name="y1", bufs=3))
            p_y2 = ctx.enter_context(tc.tile_pool(name="y2", bufs=NT))
            p_qs = ctx.enter_context(tc.tile_pool(name="qs", bufs=2))
            p_cf = ctx.enter_context(tc.tile_pool(name="cf", bufs=2))
            # PSUM budget 8 banks: pp12 6 + pp3 1 + ppt 1
            pp12 = ctx.enter_context(tc.tile_pool(name="pp12", bufs=3, space="PSUM"))
            pp3 = ctx.enter_context(tc.tile_pool(name="pp3", bufs=1, space="PSUM"))
            ppt = ctx.enter_context(tc.tile_pool(name="ppt", bufs=1, space="PSUM"))

            # ---- one-time loads (all pre-packed on host) ----
            # w1 + first obs tiles first: they gate the first matmul
            w1s = singles.tile([128, 2, HID], FP8)
            nc.sync.dma_start(out=w1s, in_=w1d)
            obs_r = obsd.rearrange("(i p) n -> p i n", p=128)
            obst = [None] * NT

            def obs_dma(k, eng=None):
                obst[k] = p_obs.tile([128, 2, BT], FP8, name=f"ob{k}", tag="obs")
                (eng or nc.sync).dma_start(
                    out=obst[k], in_=obs_r[:, :, k * BT : (k + 1) * BT]
                )

            # obs0 issues from the ACT HWDGE queue so its transfer overlaps
            # the w1 issue on sync (both gate the first real matmul)
            obs_dma(0, nc.scalar)
            obs_dma(1)
            obs_dma(2)

            # dummy matmuls while weights/obs stream in: PE-HAM needs ~3.4us
            # of sustained activity before it unthrottles 1.2->2.4 GHz, so
            # start paying that during the DMA head (transposes don't count).
            # junk is memset (no gpsimd library dep) so warmup starts early.
            junk = singles.tile([128, 128], BF16)
            nc.vector.memset(junk[:], 0.0)
            warm = pp3.tile([128, BT], F32, name="warm", tag="z3")
            for _ in range(24):
                nc.tensor.matmul(
                    warm[:, 0:128], junk[:], junk[:], start=True, stop=True
                )

            ident = singles.tile([128, 128], BF16)
            masks.make_identity(nc, ident[:])
            w2s = singles.tile([128, 4, HID], FP8)
            nc.sync.dma_start(out=w2s, in_=w2d)
            w3s = singles.tile([128, 4, 32], FP8)
            nc.sync.dma_start(out=w3s, in_=w3d)
            bs = singles.tile([128, 9], F32)
            nc.sync.dma_start(out=bs, in_=bd)

            def drain(eng, dst, src, bcol):
                # relu((z*64)/64 + b); ACT is exact, DVE computes
                # max(z*64, -64b)/64 = relu(z+b) - b (deficit pre-folded
                # into the next layer's bias on the host).
                if eng == "A":
                    bias = 0.0 if zero_bias else bs[:, bcol : bcol + 1]
                    nc.scalar.activation(
                        out=dst, in_=src, func=AF.Relu, bias=bias, scale=1.0 / WS
                    )
                else:
                    s1 = 0.0 if zero_bias else bs[:, bcol : bcol + 1]
                    nc.vector.tensor_scalar(dst, src, s1, 1.0 / WS, ALU.max, ALU.mult)


            y1_of = {}

            def L1(t):
                # layer 1: z1' = W1'.T @ obs (DoubleRow: K=256 in one pass)
                y1 = p_y1.tile([128, 4, BT], FP8, name=f"y1_{t}", tag="y1")
                ob = obst[t]
                if zero_bias:
                    for h in range(2):  # halves: m chunks (2h, 2h+1)
                        ps = pp12.tile([128, 2, BT], F32, name="ps1", tag="pp")
                        for m in (2 * h, 2 * h + 1):
                            nc.tensor.matmul(
                                ps[:, m - 2 * h, :],
                                w1s[:, :, m * 128 : (m + 1) * 128],
                                ob,
                                start=True,
                                stop=True,
                                perf_mode=PM.DoubleRow,
                            )
                        drain("D", y1[:, 2 * h : 2 * h + 2, :], ps, 0)
                else:
                    for m in range(4):
                        ps = pp12.tile([128, BT], F32, name="ps1", tag="pp1")
                        nc.tensor.matmul(
                            ps,
                            w1s[:, :, m * 128 : (m + 1) * 128],
                            ob,
                            start=True,
                            stop=True,
                            perf_mode=PM.DoubleRow,
                        )
                        drain("ADDA"[m], y1[:, m, :], ps, m)
                y1_of[t] = y1

            def L2(t):
                # layer 2: K=512 as 2 DoubleRow passes
                y1 = y1_of.pop(t)
                y2 = p_y2.tile([128, 4, BT], FP8, name=f"y2_{t}", tag="y2")
                if zero_bias:
                    for h in range(2):
                        ps = pp12.tile([128, 2, BT], F32, name="ps2", tag="pp")
                        for m in (2 * h, 2 * h + 1):
                            for c in range(2):
                                nc.tensor.matmul(
                                    ps[:, m - 2 * h, :],
                                    w2s[:, 2 * c : 2 * c + 2, m * 128 : (m + 1) * 128],
                                    y1[:, 2 * c : 2 * c + 2, :],
                                    start=(c == 0),
                                    stop=(c == 1),
                                    perf_mode=PM.DoubleRow,
                                )
                        drain("A", y2[:, 2 * h : 2 * h + 2, :], ps, 0)
                else:
                    for m in range(4):
                        ps = pp12.tile([128, BT], F32, name="ps2", tag="pp1")
                        for c in range(2):
                            nc.tensor.matmul(
                                ps,
                                w2s[:, 2 * c : 2 * c + 2, m * 128 : (m + 1) * 128],
                                y1[:, 2 * c : 2 * c + 2, :],
                                start=(c == 0),
                                stop=(c == 1),
                                perf_mode=PM.DoubleRow,
                            )
                        drain("DAAD"[m], y2[:, m, :], ps, 4 + m)
                y2r.append(y2)

            y2r = []  # current round's y2 tiles
            L1(0)
            L1(1)
            for t in range(NT):
                if t + 3 < NT:
                    obs_dma(t + 3)
                if t + 2 < NT:
                    L1(t + 2)
                L2(t)

                if t % 4 != 3:
                    continue

                # ---- layer 3 for tiles 4r..4r+3, col-tiled into one bank ----
                r = t // 4
                ps3 = pp3.tile([128, BT], F32, tag="z3")
                for kc in range(4):
                    for g in range(4):
                        nc.tensor.matmul(
                            ps3[32 * g : 32 * (g + 1), :],
                            w3s[:, kc, :],
                            y2r[g][:, kc, :],
                            start=(kc == 0),
                            stop=(kc == 3),
                            tile_position=(0, 32 * g),
                        )
                y2r = []
                # h = tanh((z3+b3)/2), so q = (1+h)/2: tanh shares an ACT
                # table with relu/exp/copy/square (sigmoid does not, and
                # mixing tables costs a 1.3us table load per switch)
                qs = p_qs.tile([128, BT], BF16, tag="qs")
                nc.scalar.activation(
                    out=qs, in_=ps3, func=AF.Tanh, bias=bs[:, 8:9], scale=0.5 / WS
                )
                # transpose whole round to batch-major: [128, (c, 32g+f)]
                pt = ppt.tile([128, 4, 4, 32], BF16, tag="pt")
                for c in range(4):
                    nc.tensor.transpose(
                        pt[:, c, :, :], qs[:, c * 128 : (c + 1) * 128], ident[:]
                    )
                hq = pt[:, :, :, 0:4]
                hp = pt[:, :, :, 4:8]

                u0b = p_cf.tile([128, 4, 4, 4], F32, tag="u0b")
                nc.sync.dma_start(out=u0b, in_=u0d[r])

                # closed form u_N = A*(u0 + w) - w with q = (1+hq)/2,
                # w = p/(2q) = (0.5 + 0.5*hp)/(1 + hq), and
                # A = (1 - 2*lr*q)^100 = 0.99^100 * exp(c1*hq + c2*hq^2)
                # (2-term log series in hq, |trunc err| < 4e-5).
                EPS = LR / (1.0 - LR)
                C1 = -100.0 * EPS
                C2 = -50.0 * EPS * EPS
                A0 = float(np.exp(100.0 * np.log(1.0 - LR)))
                oq = p_cf.tile([128, 4, 4, 4], F32, tag="oq")  # 1 + hq
                nc.scalar.activation(out=oq, in_=hq, func=AF.Copy, bias=1.0)
                np_ = p_cf.tile([128, 4, 4, 4], F32, tag="np_")  # (1+hp)/2
                nc.scalar.activation(
                    out=np_, in_=hp, func=AF.Copy, bias=0.5, scale=0.5
                )
                t1 = p_cf.tile([128, 4, 4, 4], F32, tag="t1")
                nc.scalar.activation(
                    out=t1, in_=hq, func=AF.Copy, bias=C1, scale=C2
                )
                g = p_cf.tile([128, 4, 4, 4], F32, tag="g")
                nc.vector.tensor_tensor(out=g, in0=t1, in1=hq, op=ALU.mult)
                E = p_cf.tile([128, 4, 4, 4], F32, tag="E")
                nc.scalar.activation(out=E, in_=g, func=AF.Exp)
                d = p_cf.tile([128, 4, 4, 4], F32, tag="d")
                nc.vector.reciprocal(d, oq)
                w = p_cf.tile([128, 4, 4, 4], F32, tag="w")
                nc.vector.tensor_tensor(out=w, in0=np_, in1=d, op=ALU.mult)
                s = p_cf.tile([128, 4, 4, 4], F32, tag="s")
                nc.vector.tensor_tensor(out=s, in0=u0b, in1=w, op=ALU.add)
                As = p_cf.tile([128, 4, 4, 4], F32, tag="As")  # A0*(u0+w)
                nc.vector.tensor_scalar(As, s, A0, None, ALU.mult)
                mm = p_cf.tile([128, 4, 4, 4], F32, tag="mm")
                nc.vector.tensor_tensor(out=mm, in0=E, in1=As, op=ALU.mult)
                uob = p_cf.tile([128, 4, 4, 4], F32, tag="uob")
                nc.vector.tensor_tensor(out=uob, in0=mm, in1=w, op=ALU.subtract)
                nc.sync.dma_start(out=uod[r], in_=uob)
    nc.finalize()
    return nc


def _get_nc(zero_bias: bool):
    key = ("nc", zero_bias)
    if key not in _CACHE:
        _CACHE[key] = _build_nc(zero_bias)
    return _CACHE[key]


def kernel(obs, x_init, u_init, W1, b1, W2, b2, W3, b3):
    obs = np.asarray(obs, dtype=np.float32)
    u_init = np.ascontiguousarray(np.asarray(u_init, dtype=np.float32))
    W1 = np.asarray(W1, dtype=np.float32)
    W2 = np.asarray(W2, dtype=np.float32)
    W3 = np.asarray(W3, dtype=np.float32)
    b1 = np.asarray(b1, dtype=np.float32)
    b2 = np.asarray(b2, dtype=np.float32)
    b3 = np.asarray(b3, dtype=np.float32)

    zero_bias = not (np.any(b1) or np.any(b2))

    # only columns 12:16 (q_u) and 28:32 (p_u) of the MLP head matter
    W3u = np.concatenate([W3[:, 12:16], W3[:, 28:32]], axis=1)  # [512, 8]
    b3u = np.concatenate([b3[12:16], b3[28:32]])  # [8]

    # fp8 packs; weights scaled x64 (drain scale undoes it)
    obs8 = obs.astype(NPF8)
    w1p = np.ascontiguousarray(
        (W1 * WS).astype(NPF8).reshape(2, 128, HID).transpose(1, 0, 2)
    )
    w2p = np.ascontiguousarray(
        (W2 * WS).astype(NPF8).reshape(4, 128, HID).transpose(1, 0, 2)
    )
    w3z = np.zeros((HID, 32), np.float32)
    w3z[:, :8] = W3u * WS
    w3p = np.ascontiguousarray(w3z.astype(NPF8).reshape(4, 128, 32).transpose(1, 0, 2))

    # bias pack + host-side deficit corrections for DVE-drained chunks
    if zero_bias:
        b2e = b2
        b3e = b3u
    else:
        # L1 DVE chunks m1,m2 store y1 - b1 on those features
        b2e = b2 + W2[128:384].T @ b1[128:384]
        # L2 DVE chunks m0,m3
        b3e = b3u + W3u[0:128].T @ b2e[0:128] + W3u[384:512].T @ b2e[384:512]
    bp = np.zeros((128, 9), np.float32)
    for m, e in enumerate("ADDA"):
        c = b1[m * 128 : (m + 1) * 128]
        bp[:, m] = c if e == "A" else -WS * c
    for m, e in enumerate("DAAD"):
        c = b2e[m * 128 : (m + 1) * 128]
        bp[:, 4 + m] = c if e == "A" else -WS * c
    for g in range(4):
        bp[32 * g : 32 * g + 8, 8] = 0.5 * b3e  # tanh((z+b3)/2) form

    nc = _get_nc(zero_bias)
    in_maps = []
    for i in range(NCORES):
        sl = slice(i * BPC, (i + 1) * BPC)
        u0p = (
            u_init[sl]
            .reshape(NR, 4, 4, 128, 4)  # [r, g, c, n, j]
            .transpose(0, 3, 2, 1, 4)  # [r, n, c, g, j]
            .reshape(NR, 128, 64)
        )
        in_maps.append(
            {
                "obsd": np.ascontiguousarray(obs8[sl].T),
                "w1d": w1p,
                "w2d": w2p,
                "w3d": w3p,
                "bd": bp,
                "u0d": np.ascontiguousarray(u0p),
            }
        )
    import os

    kw = {}
    if os.environ.get("BASSK_TRACE"):
        kw = {"trace": True, "tmpdir": os.environ.get("BASSK_TRACE_DIR") or None}
    res = run_bass_kernel_spmd(nc, in_maps, list(range(NCORES)), **kw)
    _CACHE["last_result"] = res
    outs = []
    for i in range(NCORES):
        uop = res.results[i]["uod"].reshape(NR, 128, 4, 4, 4)
        outs.append(uop.transpose(0, 3, 2, 1, 4).reshape(BPC, 4))
    return np.concatenate(outs, axis=0).astype(np.float32)


# revision 43
# speedup vs baseline: 1.0508x; 1.0508x over previous
"""Trainium2 Bass kernel for nn_MPCActor: MLP (256->512->512->32, relu/relu/
sigmoid) + 100 SGD steps on u, solved in closed form (u <- a*u + b with
a = 1-2*lr*q_u has exact solution u_N = a^N u0 - 0.5*(p_u/q_u)*(1 - a^N)).

Data parallel over 8 NeuronCores: batch 32768 -> 4096 rows/core, weights
replicated. All matmul operands are fp8(e4m3): obs is pre-transposed and
pre-quantized on the host to [256, 4096] per core (feature-on-partition, so
no on-chip transposes), weights are scaled x64 on the host to dodge the fp8
subnormal cliff (undone in the drain scale) and packed in the DoubleRow
[K=128, 2, M] pair layout so each matmul contracts 256 rows per pass.

Layer 3 computes only the 8 useful W3 columns (q_u, p_u), zero-padded to 32
and col-tiled via tile_position so four batch tiles land concurrently in one
PSUM bank at partition offsets 0/32/64/96; one sigmoid drains all four, and
four [128,128] bf16 PE transposes flip a whole 4-tile round to batch-major.
The closed-form update then runs once per round on [128,4,4,4] APs.

PSUM drains alternate ACT/DVE; the closed-form's SBUF-only power chain runs
on the otherwise idle GpSimd. When the MLP biases are nonzero the kernel
falls back to per-chunk drains (ACT with exact bias; DVE/GpSimd chunks use
max(z,-64b)/64 whose constant deficit is folded into the next layer's bias
on the host).
"""

import numpy as np
import ml_dtypes

import concourse.bass as bass
import concourse.mybir as mybir
import concourse.tile as tile
from concourse import bacc, masks
from concourse.bass_utils import run_bass_kernel_spmd

NCORES = 8
BATCH = 32768
BPC = BATCH // NCORES  # 4096
OBS = 256
HID = 512
BT = 512               # batch tile (matmul moving free dim)
NT = BPC // BT         # 8 batch tiles per core
NR = NT // 4           # rounds of 4 tiles for layer 3
LR = 0.01
WS = 64.0              # weight pre-scale (host), undone in drain scale
F32 = mybir.dt.float32
BF16 = mybir.dt.bfloat16
FP8 = mybir.dt.float8e4
NPF8 = ml_dtypes.float8_e4m3

_CACHE = {}


def _build_nc(zero_bias: bool):
    nc = bacc.Bacc(
        trn_type="TRN2", target_bir_lowering=False, debug=False, num_devices=NCORES
    )
    obsd = nc.declare_dram_parameter("obsd", [OBS, BPC], FP8, isOutput=False).ap()
    w1d = nc.declare_dram_parameter("w1d", [128, 2, HID], FP8, isOutput=False).ap()
    w2d = nc.declare_dram_parameter("w2d", [128, 4, HID], FP8, isOutput=False).ap()
    w3d = nc.declare_dram_parameter("w3d", [128, 4, 32], FP8, isOutput=False).ap()
    bd = nc.declare_dram_parameter("bd", [128, 9], F32, isOutput=False).ap()
    u0d = nc.declare_dram_parameter("u0d", [NR, 128, 64], F32, isOutput=False).ap()
    uod = nc.declare_dram_parameter("uod", [NR, 128, 64], F32, isOutput=True).ap()

    AF = mybir.ActivationFunctionType
    ALU = mybir.AluOpType
    PM = mybir.MatmulPerfMode

    with tile.TileContext(nc) as tc:
        from contextlib import ExitStack

        with ExitStack() as ctx:
            singles = ctx.enter_context(tc.tile_pool(name="singles", bufs=1))
            p_obs = ctx.enter_context(tc.tile_pool(name="obs", bufs=4))
            p_y1 = ctx.enter_context(tc.tile_pool(name="y1", bufs=3))
            p_y2 = ctx.enter_context(tc.tile_pool(name="y2", bufs=NT))
            p_qs = ctx.enter_context(tc.tile_pool(name="qs", bufs=2))
            p_cf = ctx.enter_context(tc.tile_pool(name="cf", bufs=2))
            # PSUM budget 8 banks: pp12 6 + pp3 1 + ppt 1
            pp12 = ctx.enter_context(tc.tile_pool(name="pp12", bufs=3, space="PSUM"))
            pp3 = ctx.enter_context(tc.tile_pool(name="pp3", bufs=1, space="PSUM"))
            ppt = ctx.enter_context(tc.tile_pool(name="ppt", bufs=1, space="PSUM"))

            # ---- one-time loads (all pre-packed on host) ----
            # w1 + first obs tiles first: they gate the first matmul
            w1s = singles.tile([128, 2, HID], FP8)
            nc.sync.dma_start(out=w1s, in_=w1d)
            obs_r = obsd.rearrange("(i p) n -> p i n", p=128)
            obst = [None] * NT

            def obs_dma(k, eng=None):
                obst[k] = p_obs.tile([128, 2, BT], FP8, name=f"ob{k}", tag="obs")
                (eng or nc.sync).dma_start(
                    out=obst[k], in_=obs_r[:, :, k * BT : (k + 1) * BT]
                )

            # obs0 issues from the ACT HWDGE queue so its transfer overlaps
            # the w1 issue on sync (both gate the first real matmul)
            obs_dma(0, nc.scalar)
            obs_dma(1)
            obs_dma(2)

            # dummy matmuls while weights/obs stream in: PE-HAM needs ~3.4us
            # of sustained activity before it unthrottles 1.2->2.4 GHz, so
            # start paying that during the DMA head (transposes don't count).
            # junk is memset (no gpsimd library dep) so warmup starts early.
            junk = singles.tile([128, 128], BF16)
            nc.vector.memset(junk[:], 0.0)
            warm = pp3.tile([128, BT], F32, name="warm", tag="z3")
            for _ in range(24):
                nc.tensor.matmul(
                    warm[:, 0:128], junk[:], junk[:], start=True, stop=True
                )

            ident = singles.tile([128, 128], BF16)
            masks.make_identity(nc, ident[:])
            w2s = singles.tile([128, 4, HID], FP8)
            nc.sync.dma_start(out=w2s, in_=w2d)
            w3s = singles.tile([128, 4, 32], FP8)
            nc.sync.dma_start(out=w3s, in_=w3d)
            bs = singles.tile([128, 9], F32)
            nc.sync.dma_start(out=bs, in_=bd)

            def drain(eng, dst, src, bcol):
                # relu((z*64)/64 + b); ACT is exact, DVE computes
                # max(z*64, -64b)/64 = relu(z+b) - b (deficit pre-folded
                # into the next layer's bias on the host).
                if eng == "A":
                    bias = 0.0 if zero_bias else bs[:, bcol : bcol + 1]
                    nc.scalar.activation(
                        out=dst, in_=src, func=AF.Relu, bias=bias, scale=1.0 / WS
                    )
                else:
                    s1 = 0.0 if zero_bias else bs[:, bcol : bcol + 1]
                    nc.vector.tensor_scalar(dst, src, s1, 1.0 / WS, ALU.max, ALU.mult)


            y1_of = {}

            def L1(t):
                # layer 1: z1' = W1'.T @ obs (DoubleRow: K=256 in one pass)
                y1 = p_y1.tile([128, 4, BT], FP8, name=f"y1_{t}", tag="y1")
                ob = obst[t]
                if zero_bias:
                    for h in range(2):  # halves: m chunks (2h, 2h+1)
                        ps = pp12.tile([128, 2, BT], F32, name="ps1", tag="pp")
                        for m in (2 * h, 2 * h + 1):
                            nc.tensor.matmul(
                                ps[:, m - 2 * h, :],
                                w1s[:, :, m * 128 : (m + 1) * 128],
                                ob,
                                start=True,
                                stop=True,
                                perf_mode=PM.DoubleRow,
                            )
                        drain("AD"[h], y1[:, 2 * h : 2 * h + 2, :], ps, 0)
                else:
                    for m in range(4):
                        ps = pp12.tile([128, BT], F32, name="ps1", tag="pp1")
                        nc.tensor.matmul(
                            ps,
                            w1s[:, :, m * 128 : (m + 1) * 128],
                            ob,
                            start=True,
                            stop=True,
                            perf_mode=PM.DoubleRow,
                        )
                        drain("ADDA"[m], y1[:, m, :], ps, m)
                y1_of[t] = y1

            def L2(t):
                # layer 2: K=512 as 2 DoubleRow passes
                y1 = y1_of.pop(t)
                y2 = p_y2.tile([128, 4, BT], FP8, name=f"y2_{t}", tag="y2")
                if zero_bias:
                    for h in range(2):
                        ps = pp12.tile([128, 2, BT], F32, name="ps2", tag="pp")
                        for m in (2 * h, 2 * h + 1):
                            for c in range(2):
                                nc.tensor.matmul(
                                    ps[:, m - 2 * h, :],
                                    w2s[:, 2 * c : 2 * c + 2, m * 128 : (m + 1) * 128],
                                    y1[:, 2 * c : 2 * c + 2, :],
                                    start=(c == 0),
                                    stop=(c == 1),
                                    perf_mode=PM.DoubleRow,
                                )
                        drain("AD"[h], y2[:, 2 * h : 2 * h + 2, :], ps, 0)
                else:
                    for m in range(4):
                        ps = pp12.tile([128, BT], F32, name="ps2", tag="pp1")
                        for c in range(2):
                            nc.tensor.matmul(
                                ps,
                                w2s[:, 2 * c : 2 * c + 2, m * 128 : (m + 1) * 128],
                                y1[:, 2 * c : 2 * c + 2, :],
                                start=(c == 0),
                                stop=(c == 1),
                                perf_mode=PM.DoubleRow,
                            )
                        drain("DAAD"[m], y2[:, m, :], ps, 4 + m)
                y2r.append(y2)

            y2r = []  # current round's y2 tiles
            L1(0)
            L1(1)
            for t in range(NT):
                if t + 3 < NT:
                    obs_dma(t + 3)
                if t + 2 < NT:
                    L1(t + 2)
                L2(t)

                if t % 4 != 3:
                    continue

                # ---- layer 3 for tiles 4r..4r+3, col-tiled into one bank ----
                r = t // 4
                ps3 = pp3.tile([128, BT], F32, tag="z3")
                for kc in range(4):
                    for g in range(4):
                        nc.tensor.matmul(
                            ps3[32 * g : 32 * (g + 1), :],
                            w3s[:, kc, :],
                            y2r[g][:, kc, :],
                            start=(kc == 0),
                            stop=(kc == 3),
                            tile_position=(0, 32 * g),
                        )
                y2r = []
                # h = tanh((z3+b3)/2), so q = (1+h)/2: tanh shares an ACT
                # table with relu/exp/copy/square (sigmoid does not, and
                # mixing tables costs a 1.3us table load per switch)
                qs = p_qs.tile([128, BT], BF16, tag="qs")
                nc.scalar.activation(
                    out=qs, in_=ps3, func=AF.Tanh, bias=bs[:, 8:9], scale=0.5 / WS
                )
                # transpose whole round to batch-major: [128, (c, 32g+f)]
                pt = ppt.tile([128, 4, 4, 32], BF16, tag="pt")
                for c in range(4):
                    nc.tensor.transpose(
                        pt[:, c, :, :], qs[:, c * 128 : (c + 1) * 128], ident[:]
                    )
                hq = pt[:, :, :, 0:4]
                hp = pt[:, :, :, 4:8]

                u0b = p_cf.tile([128, 4, 4, 4], F32, tag="u0b")
                nc.sync.dma_start(out=u0b, in_=u0d[r])

                # closed form u_N = A*(u0 + w) - w with q = (1+hq)/2,
                # w = p/(2q) = (0.5 + 0.5*hp)/(1 + hq), and
                # A = (1 - 2*lr*q)^100 = 0.99^100 * exp(c1*hq + c2*hq^2)
                # (2-term log series in hq, |trunc err| < 4e-5).
                EPS = LR / (1.0 - LR)
                C1 = -100.0 * EPS
                C2 = -50.0 * EPS * EPS
                A0 = float(np.exp(100.0 * np.log(1.0 - LR)))
                oq = p_cf.tile([128, 4, 4, 4], F32, tag="oq")  # 1 + hq
                nc.scalar.activation(out=oq, in_=hq, func=AF.Copy, bias=1.0)
                np_ = p_cf.tile([128, 4, 4, 4], F32, tag="np_")  # (1+hp)/2
                nc.scalar.activation(
                    out=np_, in_=hp, func=AF.Copy, bias=0.5, scale=0.5
                )
                t1 = p_cf.tile([128, 4, 4, 4], F32, tag="t1")
                nc.vector.tensor_scalar(t1, hq, C2, C1, ALU.mult, ALU.add)
                g = p_cf.tile([128, 4, 4, 4], F32, tag="g")
                nc.vector.tensor_tensor(out=g, in0=t1, in1=hq, op=ALU.mult)
                E = p_cf.tile([128, 4, 4, 4], F32, tag="E")
                nc.scalar.activation(out=E, in_=g, func=AF.Exp)
                d = p_cf.tile([128, 4, 4, 4], F32, tag="d")
                nc.vector.reciprocal(d, oq)
                w = p_cf.tile([128, 4, 4, 4], F32, tag="w")
                nc.vector.tensor_tensor(out=w, in0=np_, in1=d, op=ALU.mult)
                s = p_cf.tile([128, 4, 4, 4], F32, tag="s")
                nc.vector.tensor_tensor(out=s, in0=u0b, in1=w, op=ALU.add)
                As = p_cf.tile([128, 4, 4, 4], F32, tag="As")  # A0*(u0+w)
                nc.vector.tensor_scalar(As, s, A0, None, ALU.mult)
                mm = p_cf.tile([128, 4, 4, 4], F32, tag="mm")
                nc.vector.tensor_tensor(out=mm, in0=E, in1=As, op=ALU.mult)
                uob = p_cf.tile([128, 4, 4, 4], F32, tag="uob")
                nc.vector.tensor_tensor(out=uob, in0=mm, in1=w, op=ALU.subtract)
                nc.sync.dma_start(out=uod[r], in_=uob)
    nc.finalize()
    return nc


def _get_nc(zero_bias: bool):
    key = ("nc", zero_bias)
    if key not in _CACHE:
        _CACHE[key] = _build_nc(zero_bias)
    return _CACHE[key]


def kernel(obs, x_init, u_init, W1, b1, W2, b2, W3, b3):
    obs = np.asarray(obs, dtype=np.float32)
    u_init = np.ascontiguousarray(np.asarray(u_init, dtype=np.float32))
    W1 = np.asarray(W1, dtype=np.float32)
    W2 = np.asarray(W2, dtype=np.float32)
    W3 = np.asarray(W3, dtype=np.float32)
    b1 = np.asarray(b1, dtype=np.float32)
    b2 = np.asarray(b2, dtype=np.float32)
    b3 = np.asarray(b3, dtype=np.float32)

    zero_bias = not (np.any(b1) or np.any(b2))

    # only columns 12:16 (q_u) and 28:32 (p_u) of the MLP head matter
    W3u = np.concatenate([W3[:, 12:16], W3[:, 28:32]], axis=1)  # [512, 8]
    b3u = np.concatenate([b3[12:16], b3[28:32]])  # [8]

    # fp8 packs; weights scaled x64 (drain scale undoes it)
    obs8 = obs.astype(NPF8)
    w1p = np.ascontiguousarray(
        (W1 * WS).astype(NPF8).reshape(2, 128, HID).transpose(1, 0, 2)
    )
    w2p = np.ascontiguousarray(
        (W2 * WS).astype(NPF8).reshape(4, 128, HID).transpose(1, 0, 2)
    )
    w3z = np.zeros((HID, 32), np.float32)
    w3z[:, :8] = W3u * WS
    w3p = np.ascontiguousarray(w3z.astype(NPF8).reshape(4, 128, 32).transpose(1, 0, 2))

    # bias pack + host-side deficit corrections for DVE-drained chunks
    if zero_bias:
        b2e = b2
        b3e = b3u
    else:
        # L1 DVE chunks m1,m2 store y1 - b1 on those features
        b2e = b2 + W2[128:384].T @ b1[128:384]
        # L2 DVE chunks m0,m3
        b3e = b3u + W3u[0:128].T @ b2e[0:128] + W3u[384:512].T @ b2e[384:512]
    bp = np.zeros((128, 9), np.float32)
    for m, e in enumerate("ADDA"):
        c = b1[m * 128 : (m + 1) * 128]
        bp[:, m] = c if e == "A" else -WS * c
    for m, e in enumerate("DAAD"):
        c = b2e[m * 128 : (m + 1) * 128]
        bp[:, 4 + m] = c if e == "A" else -WS * c
    for g in range(4):
        bp[32 * g : 32 * g + 8, 8] = 0.5 * b3e  # tanh((z+b3)/2) form

    nc = _get_nc(zero_bias)
    in_maps = []
    for i in range(NCORES):
        sl = slice(i * BPC, (i + 1) * BPC)
        u0p = (
            u_init[sl]
            .reshape(NR, 4, 4, 128, 4)  # [r, g, c, n, j]
            .transpose(0, 3, 2, 1, 4)  # [r, n, c, g, j]
            .reshape(NR, 128, 64)
        )
        in_maps.append(
            {
                "obsd": np.ascontiguousarray(obs8[sl].T),
                "w1d": w1p,
                "w2d": w2p,
                "w3d": w3p,
                "bd": bp,
                "u0d": np.ascontiguousarray(u0p),
            }
        )
    import os

    kw = {}
    if os.environ.get("BASSK_TRACE"):
        kw = {"trace": True, "tmpdir": os.environ.get("BASSK_TRACE_DIR") or None}
    res = run_bass_kernel_spmd(nc, in_maps, list(range(NCORES)), **kw)
    _CACHE["last_result"] = res
    outs = []
    for i in range(NCORES):
        uop = res.results[i]["uod"].reshape(NR, 128, 4, 4, 4)
        outs.append(uop.transpose(0, 3, 2, 1, 4).reshape(BPC, 4))
    return np.concatenate(outs, axis=0).astype(np.float32)
